# revision 3
# baseline (speedup 1.0000x reference)
# Trainium2 Bass kernel v7 for nn_AttentionCombiner — collective-free.
#
# Sharding: 8 cores = 4 batches x 2 q-halves (all 8 heads per core).
# Host side reorders the keys per core so l-blocks 0..7 are the core's OWN
# q-rows and 8..15 the peer's (identity for even cores). Then softmax
# row-sums are computed fully locally:
#   - own half  (Sum_{l in my half} S[l,q] = Sum_{q'} S[q,q'] by symmetry):
#     DVE tensor_reduce over the exp'd S tiles 0..7.
#   - peer half: column sums of S tiles 8..15 via ap_size-1 ones-matmuls on
#     PE (8 per tile, stationary=S chunk, moving=ones[128,1]) accumulated in
#     a single PSUM bank. ~53ns each: 64/head adds ~3.4us/head to PE.
# No collective, no DRAM bounce, no cross-engine latency chains: every
# dependency is local and short.
#
# Engine budget/head (TimelineSim): PE 18.8us (bottleneck: MM1+MM2 13.6,
# FC 1.7, ones 3.4), ACT 16.6 (16 exps, no accum_out), DVE ~9.5 (8 reduces
# + rsum add + recip), Pool ~8 (FC combine STTs + outT copy).
# FC for head h runs during head h+1 (recip is ready at head h's end).

import numpy as np
import ml_dtypes

N, S, D_IN, HEADS = 4, 2048, 512, 8
HEAD_DIM = 128          # 2*D_IN // HEADS
DF = 2 * D_IN           # 1024 combined features
QH = S // 2             # 1024 rows per core
NB = S // 128           # 16 l-blocks
QB = QH // 128          # 8 q-blocks per core
ISQ = 1.0 / float(np.sqrt(np.float32(HEAD_DIM)))

_CACHED_NC = None


def _build_nc(no_collective=False):
    # no_collective kept for API compat; this kernel has no collectives.
    import concourse.mybir as mybir
    import concourse.tile as tile
    from concourse import bacc
    from concourse.bass import ts

    f32 = mybir.dt.float32
    bf16 = mybir.dt.bfloat16
    Exp = mybir.ActivationFunctionType.Exp
    mult = mybir.AluOpType.mult
    add = mybir.AluOpType.add
    AxX = mybir.AxisListType.X

    nc = bacc.Bacc("TRN2", target_bir_lowering=False, debug=False, num_devices=8)

    xt = nc.dram_tensor("xt", [DF, S], bf16, kind="ExternalInput")      # X^T, l-permuted
    xtq = nc.dram_tensor("xtq", [DF, QH], bf16, kind="ExternalInput")   # X^T my-half cols
    x = nc.dram_tensor("x", [S, DF], bf16, kind="ExternalInput")        # X, l-permuted
    w = nc.dram_tensor("w", [DF, D_IN], bf16, kind="ExternalInput")     # W_out
    bias = nc.dram_tensor("bias", [128, D_IN], f32, kind="ExternalInput")
    ones = nc.dram_tensor("ones", [128, 1], bf16, kind="ExternalInput")
    out = nc.dram_tensor("out", [QH, D_IN], f32, kind="ExternalOutput")

    LAST = HEADS - 1

    with tile.TileContext(nc) as tc:
        with (
            tc.tile_pool(name="persist", bufs=1) as pers,
            tc.tile_pool(name="spool", bufs=8) as spool,
            tc.tile_pool(name="outp", bufs=3) as outp,
            tc.tile_pool(name="rpool", bufs=3) as rpool,
            tc.tile_pool(name="psE", bufs=2, space="PSUM") as psE,
            tc.tile_pool(name="psO", bufs=1, space="PSUM") as psO,
            tc.tile_pool(name="psFC", bufs=1, space="PSUM") as psFC,
            tc.tile_pool(name="psR", bufs=1, space="PSUM") as psR,
        ):
            # ---- persistent SBUF data ----
            xt_sb = pers.tile([128, HEADS, S], bf16, name="xt_sb")
            xtq_sb = pers.tile([128, HEADS, QH], bf16, name="xtq_sb")
            x_sb = pers.tile([128, NB, DF], bf16, name="x_sb")
            w_sb = pers.tile([128, HEADS, D_IN], bf16, name="w_sb")
            bias_sb = pers.tile([128, D_IN], f32, name="bias_sb")
            ones_sb = pers.tile([128, 1], bf16, name="ones_sb")

            xt_r = xt.ap().rearrange("(h p) s -> p h s", p=128)
            xtq_r = xtq.ap().rearrange("(h p) s -> p h s", p=128)
            x_r = x.ap().rearrange("(o p) f -> p o f", p=128)
            w_r = w.ap().rearrange("(h p) o -> p h o", p=128)

            # Startup loads. HWDGE costs ~0.6us per descriptor, so COUNT
            # matters: coarse chunks, few DMAs, small/urgent tensors first.
            nc.sync.dma_start(xtq_sb[:, 0, 0:512], xtq_r[:, 0, 0:512])
            nc.sync.dma_start(xt_sb[:, 0, 0:512], xt_r[:, 0, 0:512])
            nc.sync.dma_start(xtq_sb[:, 0, 512:1024], xtq_r[:, 0, 512:1024])
            nc.sync.dma_start(xt_sb[:, 0, 512:1024], xt_r[:, 0, 512:1024])
            nc.sync.dma_start(xt_sb[:, 0, 1024:2048], xt_r[:, 0, 1024:2048])
            nc.sync.dma_start(ones_sb[:], ones.ap())
            nc.sync.dma_start(bias_sb[:], bias.ap())
            for i in range(0, NB, 4):
                nc.sync.dma_start(x_sb[:, i : i + 4, :], x_r[:, i : i + 4, :])
            for h in range(1, HEADS):
                nc.sync.dma_start(xt_sb[:, h, :], xt_r[:, h, :])
                nc.sync.dma_start(xtq_sb[:, h, :], xtq_r[:, h, :])
            nc.sync.dma_start(w_sb[:], w_r[:])

            # fc accumulators, persist across heads
            accs = []
            for j in range(QB):
                a = pers.tile([128, D_IN], f32, name=f"acc{j}")
                accs.append(a)

            # fc_steps[h]: QB combine steps, popped during head h+1
            fc_steps = {}

            def make_fc_steps(h, outT, recip):
                steps = []
                ring = []
                for j in range(QB):
                    def step(h=h, j=j, outT=outT, recip=recip, eng=None,
                             ring=ring):
                        if h == LAST:
                            # 4-slot ring over the freed psE banks
                            if j % 2 == 0:
                                ring.append(psE.tile([128, QH], f32,
                                                     tag="pse", name="pfc"))
                            t = ring[(j // 2) % 2] if len(ring) >= 2 else ring[0]
                            pfc = t[:, (j % 2) * D_IN : (j % 2 + 1) * D_IN]
                        else:
                            pfc = psFC.tile([128, D_IN], f32, tag="pfc", name="pfc")
                        nc.tensor.matmul(pfc[:], outT[:, ts(j, 128)],
                                         w_sb[:, h, :], start=True, stop=True)
                        eng = nc.vector
                        if h == 0:
                            eng.scalar_tensor_tensor(
                                accs[j][:], pfc[:], recip[:, j : j + 1],
                                bias_sb[:], mult, add)
                        else:
                            eng.scalar_tensor_tensor(
                                accs[j][:], pfc[:], recip[:, j : j + 1],
                                accs[j][:], mult, add)
                        if h == LAST:
                            nc.sync.dma_start(out.ap()[ts(j, 128), :], accs[j][:])
                    steps.append(step)
                fc_steps[h] = steps

            for h in range(HEADS):
                racc = pers.tile([128, QB], f32, name=f"racc{h}")
                psr = psR.tile([128, QB], f32, tag="psr", name="psr")
                pso = psO.tile([128, QH], f32, tag="pso", name="pso")
                pse_tiles = {}

                def mm1(i, h=h, pse_tiles=pse_tiles):
                    pse = psE.tile([128, QH], f32, tag="pse", name="pse")
                    lhs1 = xt_sb[:, h, ts(i, 128)]
                    nc.tensor.matmul(pse[:, 0:512], lhs1, xtq_sb[:, h, 0:512],
                                     start=True, stop=True)
                    nc.tensor.matmul(pse[:, 512:1024], lhs1,
                                     xtq_sb[:, h, 512:1024],
                                     start=True, stop=True)
                    pse_tiles[i] = pse

                mm1(0)
                mm1(1)
                for i in range(NB):
                    pse = pse_tiles.pop(i)
                    s_i = spool.tile([128, QH], bf16, tag="s", name="s_i")
                    # own-half rowsums: DVE reduce, except the last own tile
                    # which uses ACT accum_out to keep DVE under its window
                    nc.scalar.activation(
                        s_i[:], pse[:], Exp, bias=0.0, scale=ISQ,
                        accum_out=racc[:, i : i + 1] if i == QB - 1 else None)
                    if i + 2 < NB:
                        mm1(i + 2)
                    if i < QB - 1:
                        nc.vector.tensor_reduce(
                            racc[:, i : i + 1], s_i[:], AxX, add)
                    lhs2 = x_sb[:, i, ts(h, 128)]
                    nc.tensor.matmul(pso[:, 0:512], lhs2, s_i[:, 0:512],
                                     start=(i == 0), stop=(i == NB - 1))
                    nc.tensor.matmul(pso[:, 512:1024], lhs2, s_i[:, 512:1024],
                                     start=(i == 0), stop=(i == NB - 1))
                    if i >= QB:
                        # peer-half rowsums: column sums via ones-matmuls,
                        # accumulated across tiles QB..NB-1 in PSUM
                        for j in range(QB):
                            nc.tensor.matmul(
                                psr[:, j : j + 1], s_i[:, ts(j, 128)],
                                ones_sb[:], start=(i == QB),
                                stop=(i == NB - 1), skip_group_check=True)
                    # fc work of head h-1, one step per iteration in the
                    # second half-head where DVE has slack
                    if h >= 1 and QB <= i and i - QB < len(fc_steps[h - 1]):
                        fc_steps[h - 1][i - QB]()

                # outT copy (DVE: the only engine that may read PSUM
                # besides ACT/PE), split so the drain can start early
                outT = outp.tile([128, QH], bf16, tag="outT", name="outT")
                nc.vector.tensor_copy(outT[:, 0:512], pso[:, 0:512])
                nc.vector.tensor_copy(outT[:, 512:1024], pso[:, 512:1024])

                # complete rowsums + reciprocal, all local, at head end
                rsum = rpool.tile([128, QB], f32, tag="rsum", name="rsum")
                nc.vector.tensor_tensor(rsum[:], racc[:], psr[:], add)
                recip = rpool.tile([128, QB], f32, tag="recip", name="recip")
                nc.vector.reciprocal(recip[:], rsum[:])

                make_fc_steps(h, outT, recip)

            # drain the last head's fc
            for step in fc_steps[LAST]:
                step()

    nc.compile()
    return nc


def _get_nc():
    global _CACHED_NC
    if _CACHED_NC is None:
        _CACHED_NC = _build_nc()
    return _CACHED_NC


def _in_maps(output1, output2, W_out, b_out):
    bf = ml_dtypes.bfloat16
    X = np.concatenate([np.asarray(output1), np.asarray(output2)], axis=2)
    Xb = X.astype(bf)
    Wb = np.ascontiguousarray(np.asarray(W_out).astype(bf))
    bias_full = np.ascontiguousarray(
        np.broadcast_to(np.asarray(b_out).astype(np.float32), (128, D_IN)))
    ones_t = np.ones((128, 1), bf)

    in_maps = []
    for c in range(8):
        n, half = c // 2, c % 2
        Xn = np.asarray(Xb[n])                      # [S, DF]
        if half == 1:
            # key reorder: my q-half rows first (identity for even cores)
            Xn = np.concatenate([Xn[QH:], Xn[:QH]], axis=0)
        Xn = np.ascontiguousarray(Xn)
        XTn = np.ascontiguousarray(Xn.T)            # [DF, S] (l-permuted)
        # my q columns: after the permutation they are always cols 0..QH
        in_maps.append({
            "x": Xn,
            "xt": XTn,
            "xtq": np.ascontiguousarray(XTn[:, 0:QH]),
            "w": Wb,
            "bias": bias_full,
            "ones": ones_t,
        })
    return in_maps


def kernel(output1, output2, W_out, b_out):
    from concourse.bass_utils import run_bass_kernel_spmd

    in_maps = _in_maps(output1, output2, W_out, b_out)
    nc = _get_nc()
    res = run_bass_kernel_spmd(nc, in_maps, core_ids=list(range(8)))

    full = np.empty((N, S, D_IN), np.float32)
    for c in range(8):
        n, half = c // 2, c % 2
        full[n, half * QH : (half + 1) * QH, :] = res.results[c]["out"]
    return full
